# revision 22
# baseline (speedup 1.0000x reference)
"""BERT self-attention (B=4, S=2048, H=1024, 16 heads x 64) on 8 TRN2 NeuronCores.

Sharding: data-parallel over batch (4) x tensor-parallel over head-groups (2).
Core c handles batch c//2 and heads [8*(c%2), 8*(c%2)+8). No cross-core comm.

Per-core kernel (fp16 matmuls, f32 PSUM):
  - host pre-casts x and W to fp16; x is transposed during load via XBAR
    DMA-transpose directly into SBUF (no PE transposes, no casts).
  - QKV projections fp16; Q/K stored fp16 head-paired [128, hp, S];
    V' stored fp16 [128, kc, head, 66] with a ones column (softmax denom).
  - scores: fp16, K=64 per head, two heads packed per PE pass via partition
    halves; PSUM [128, 2, 512] per (hp, qm, kc).
  - exp: split ACT (true exp, scale=1/8, bias calibrated) / DVE (uint16
    Schraudolph: i = round(184.664*s + B16), bitcast fp16 ~ exp(s/8)*2^-2.79).
    fp16's wide window needs no clamping for |logit| <= 8.5.
  - ctx: V'-stationary fp16, accumulated over kc in PSUM [65, 2, 512];
    ctx matmuls are emitted 2 kc behind scores so exp latency is hidden.
  - output: unnormalized ctx^T [65, 2048] per head in fp16; host does the
    denominator divide + transpose.
"""

import os
import sys
import types

sys.path.insert(0, "/opt/trn_rl_repo")

import numpy as np

import concourse.bass as bass
import concourse.tile as tile
from concourse import bacc, mybir
from concourse.bass_utils import run_bass_kernel_spmd

B, S, H = 4, 2048, 1024
NH, HD = 16, 64
NCORES = 8
HPC = NH // 2                 # 8 heads per core
HG = HPC * HD                 # 512 per-core head width
P = 128
N_HP = 4
N_QM = 4
QM = 512
N_KC = 16
CTX_LAG = 2                   # kc lag of ctx matmuls behind scores

FP32 = mybir.dt.float32
FP16 = mybir.dt.float16
U16 = mybir.dt.uint16
DTW = mybir.dt.float16        # working dtype for matmul operands

# --- exp constants -------------------------------------------------------
# DVE Schraudolph on fp16 bits: i = round(A16*s + B16), et = bitcast fp16(i)
#   = 2^((i-15360)/1024) ~= exp(s/8) * 2^((B16-15360)/1024) * saw(f)
A16 = 0.125 * 1024 * 1.4426950408889634   # 184.664...
B16 = 12506.0                             # 12544 - 38 (sawtooth centering)


def _calibrate():
    # mean multiplicative factor of the fp16 schraudolph vs true exp,
    # weighted by the contribution to the softmax denominator (~phi(L)*e^L)
    L = np.linspace(-6.0, 6.0, 48001)
    s = 8.0 * L
    i = np.clip(np.round(A16 * s + B16), 0, 65535)
    val = 2.0 ** ((i - 15360.0) / 1024.0)
    g_nom = 2.0 ** ((B16 - 15360.0) / 1024.0)
    ratio = val / (np.exp(L) * g_nom)
    w = np.exp(-0.5 * L * L) * np.exp(L)
    m = float((ratio * w).sum() / w.sum())
    return float(np.log(g_nom * m))


ACT_BIAS = _calibrate()
OUT_SCALE = float(np.exp(ACT_BIAS))

# exp engine per kc index (A=ACT true exp, D=DVE schraudolph)
EXP_PATTERN = os.environ.get("EXP_PATTERN", "AADADAADADAADADA")


def _ensure_profile_hook():
    try:
        from antenv.axon_hooks import get_axon_ntff_profile_hook  # noqa: F401
        return
    except ImportError:
        pass
    try:
        from trn_agent_boot.trn_boot import _ntff_profile_via_ctypes
    except ImportError:
        return
    hook = _ntff_profile_via_ctypes("/opt/axon/libaxon_pjrt.so")
    mod = types.ModuleType("antenv.axon_hooks")
    mod.get_axon_ntff_profile_hook = lambda: hook
    mod.set_axon_ntff_profile_hook = lambda h: None
    sys.modules["antenv.axon_hooks"] = mod


def build():
    nc = bacc.Bacc("TRN2", target_bir_lowering=False, debug=False,
                   num_devices=NCORES)

    x_d = nc.declare_dram_parameter("x", [S, H], FP16, isOutput=False)
    wq_d = nc.declare_dram_parameter("wq", [H, HG], FP16, isOutput=False)
    wk_d = nc.declare_dram_parameter("wk", [H, HG], FP16, isOutput=False)
    wv_d = nc.declare_dram_parameter("wv", [H, HG], FP16, isOutput=False)
    ball_d = nc.declare_dram_parameter("ball", [3, HG], FP32, isOutput=False)
    out_d = nc.declare_dram_parameter("out", [HPC, HD + 1, S], FP16,
                                      isOutput=True)
    if os.environ.get("DBG"):
        nc._dbg_xT = nc.declare_dram_parameter("dbg_xT", [P, 8, S], FP16,
                                               isOutput=True)
        nc._dbg_qT = nc.declare_dram_parameter("dbg_qT", [P, N_HP, S], FP16,
                                               isOutput=True)
        nc._dbg_vp = nc.declare_dram_parameter("dbg_vp", [P, N_KC, HPC, HD + 2],
                                               FP16, isOutput=True)

    with tile.TileContext(nc) as tc:
        _build_body(nc, tc, x_d, (wq_d, wk_d, wv_d), ball_d, out_d)

    nc.finalize()
    return nc


def _build_body(nc, tc, x_d, w_d, ball_d, out_d):
    wq_d, wk_d, wv_d = w_d

    import contextlib
    ctx = contextlib.ExitStack()
    with ctx:
        const = ctx.enter_context(tc.tile_pool(name="const", bufs=1))
        big = ctx.enter_context(tc.tile_pool(name="big", bufs=1))
        expp = ctx.enter_context(tc.tile_pool(name="expp", bufs=4))
        epil = ctx.enter_context(tc.tile_pool(name="epil", bufs=2))
        ps_sc = ctx.enter_context(
            tc.tile_pool(name="ps_sc", bufs=2, space="PSUM"))
        ps_ctx = ctx.enter_context(
            tc.tile_pool(name="ps_ctx", bufs=2, space="PSUM"))

        # ---- constants + big tensors; DMA emission order matters: the
        # framework recycles DMA semaphores with reuse distance ~5, so each
        # DMA is emitted >=5 slots after anything it must not wait on.
        abias = const.tile([P, 1], FP32)
        nc.gpsimd.memset(abias, ACT_BIAS)
        xT = big.tile([P, 8, S], DTW, tag="xT")
        w16 = {}
        for name, wd in (("q", wq_d), ("k", wk_d), ("v", wv_d)):
            w16[name] = big.tile([P, 8, HG], DTW, tag=f"w{name}",
                                 name=f"w{name}")
        qT = big.tile([P, N_HP, S], DTW, tag="qT")
        kT = big.tile([P, N_HP, S], DTW, tag="kT")
        vp = big.tile([P, N_KC, HPC, HD + 2], DTW, tag="vp")
        nc.gpsimd.memset(vp[:, :, :, HD:HD + 2], 1.0)  # denom col + pad

        def load_w(name, wd):
            nc.gpsimd.dma_start(
                out=w16[name], in_=wd.ap().rearrange("(o p) m -> p o m", p=P))

        # transposes split across the two HWDGE rings (SP + ACT)
        for c in range(8):
            (nc.sync if c % 2 == 0 else nc.scalar).dma_start_transpose(
                out=xT[:, c, :], in_=x_d.ap()[:, c * P:(c + 1) * P])
        load_w("q", wq_d)
        ball = const.tile([P, 3, N_HP], FP32)
        nc.gpsimd.dma_start(
            out=ball, in_=ball_d.ap().rearrange("w (o p) -> p w o", p=P))
        bqT = ball[:, 0, :]
        bkT = ball[:, 1, :]
        bv1 = const.tile([1, HG], FP32)
        nc.gpsimd.dma_start(out=bv1, in_=ball_d.ap()[2:3, :])
        bvb = const.tile([P, HG], FP32)
        nc.gpsimd.partition_broadcast(out_ap=bvb, in_ap=bv1)
        load_w("k", wk_d)
        load_w("v", wv_d)

        # ---- projections ----------------------------------------------
        def proj_qk(w_name, dst, bias, mt):
            for ncb in range(2):
                ps = ps_sc.tile([P, 2, QM], FP32, tag="sc",
                                name=f"p{w_name}{mt}{ncb}")
                for j in range(2):
                    nci = 2 * ncb + j
                    for ci in range(8):
                        hb = (ci + 2 * nci + 3 * mt) % 8
                        nc.tensor.matmul(
                            ps[:, j, :],
                            lhsT=w16[w_name][:, hb, mt * P:(mt + 1) * P],
                            rhs=xT[:, hb, nci * QM:(nci + 1) * QM],
                            start=(ci == 0), stop=(ci == 7),
                        )
                if (mt + ncb) % 2 == 0:
                    nc.vector.tensor_scalar_add(
                        out=dst[:, mt, 2 * ncb * QM:(2 * ncb + 2) * QM],
                        in0=ps.rearrange("p a b -> p (a b)"),
                        scalar1=bias[:, mt:mt + 1],
                    )
                else:
                    nc.scalar.activation(
                        out=dst[:, mt, 2 * ncb * QM:(2 * ncb + 2) * QM],
                        in_=ps.rearrange("p a b -> p (a b)"),
                        func=mybir.ActivationFunctionType.Identity,
                        bias=bias[:, mt:mt + 1],
                    )

        def proj_v(st_base):
            for st in range(st_base, st_base + 4):
                ps = ps_sc.tile([P, 2, QM], FP32, tag="sc", name=f"pv{st}")
                for ci in range(8):
                    hb = (ci + st) % 8
                    nc.tensor.matmul(
                        ps[:, 0, :],
                        lhsT=xT[:, hb, st * P:(st + 1) * P],
                        rhs=w16["v"][:, hb, :],
                        start=(ci == 0), stop=(ci == 7),
                    )
                nc.vector.scalar_tensor_tensor(
                    out=vp[:, st, :, 0:HD],
                    in0=ps[:, 0, :].rearrange("p (h d) -> p h d", h=HPC),
                    scalar=1.0,
                    in1=bvb.rearrange("p (h d) -> p h d", h=HPC),
                    op0=mybir.AluOpType.mult,
                    op1=mybir.AluOpType.add,
                )

        for grp in range(4):
            proj_qk("q", qT, bqT, grp)
            proj_qk("k", kT, bkT, grp)
            proj_v(4 * grp)

        # ---- attention -------------------------------------------------
        def emit_ctx(cps, ets, hp, kc):
            for hh in range(2):
                nc.tensor.matmul(
                    cps[:, hh, :],
                    lhsT=vp[:, kc, 2 * hp + hh, 0:HD + 1],
                    rhs=ets[kc][:, hh, :],
                    start=(kc == 0), stop=(kc == N_KC - 1),
                )

        def attention(hp, qm):
            cps = ps_ctx.tile([HD + 1, 2, QM], FP32, tag="ctx",
                              name=f"ctx{hp}{qm}")
            ets = {}
            for kc in range(N_KC):
                sc = ps_sc.tile([P, 2, QM], FP32, tag="sc",
                                name=f"sc{hp}{qm}{kc}")
                for hh in range(2):
                    psl = slice(hh * HD, (hh + 1) * HD)
                    nc.tensor.matmul(
                        sc[:, hh, :],
                        lhsT=kT[psl, hp, kc * P:(kc + 1) * P],
                        rhs=qT[psl, hp, qm * QM:(qm + 1) * QM],
                        start=True, stop=True,
                    )
                et = expp.tile([P, 2, QM], FP16, tag="et",
                               name=f"et{hp}{qm}{kc}")
                ets[kc] = et
                if EXP_PATTERN[kc] == "A":
                    nc.scalar.activation(
                        out=et, in_=sc,
                        func=mybir.ActivationFunctionType.Exp,
                        scale=0.125, bias=abias,
                    )
                else:
                    nc.vector.tensor_scalar(
                        out=et.bitcast(U16), in0=sc,
                        scalar1=A16, scalar2=B16,
                        op0=mybir.AluOpType.mult, op1=mybir.AluOpType.add,
                    )
                if kc >= CTX_LAG:
                    emit_ctx(cps, ets, hp, kc - CTX_LAG)
            for kc in range(N_KC - CTX_LAG, N_KC):
                emit_ctx(cps, ets, hp, kc)
            csb = epil.tile([HD + 1, 2, QM], FP16, tag="csb",
                            name=f"csb{hp}{qm}")
            nc.scalar.copy(out=csb[:, 0, :], in_=cps[:, 0, :])
            nc.vector.tensor_copy(out=csb[:, 1, :], in_=cps[:, 1, :])
            for hh in range(2):
                nc.gpsimd.dma_start(
                    out=out_d.ap()[2 * hp + hh, :, qm * QM:(qm + 1) * QM],
                    in_=csb[:, hh, :],
                )

        for hp in range(N_HP):
            for qm in range(N_QM):
                attention(hp, qm)
        if os.environ.get("DBG"):
            nc.sync.dma_start(out=nc._dbg_xT.ap(), in_=xT)
            nc.sync.dma_start(out=nc._dbg_qT.ap(), in_=qT)
            nc.sync.dma_start(out=nc._dbg_vp.ap(), in_=vp)


_NC_CACHE = None


def _get_nc():
    global _NC_CACHE
    if _NC_CACHE is None:
        _NC_CACHE = build()
    return _NC_CACHE


def make_in_maps(hidden_states, Wq, bq, Wk, bk, Wv, bv):
    x16 = np.asarray(hidden_states, dtype=np.float16)
    w16 = {k: np.asarray(v, dtype=np.float16)
           for k, v in (("q", Wq), ("k", Wk), ("v", Wv))}
    ball = np.stack([np.asarray(bq, dtype=np.float32),
                     np.asarray(bk, dtype=np.float32),
                     np.asarray(bv, dtype=np.float32)])
    in_maps = []
    for c in range(NCORES):
        b, g = c // 2, c % 2
        sl = slice(g * HG, (g + 1) * HG)
        in_maps.append({
            "x": np.ascontiguousarray(x16[b]),
            "wq": np.ascontiguousarray(w16["q"][:, sl]),
            "wk": np.ascontiguousarray(w16["k"][:, sl]),
            "wv": np.ascontiguousarray(w16["v"][:, sl]),
            "ball": np.ascontiguousarray(ball[:, sl]),
        })
    return in_maps


def run(in_maps, trace=False):
    _ensure_profile_hook()
    nc = _get_nc()
    return run_bass_kernel_spmd(nc, in_maps, list(range(NCORES)), trace=trace)


def unshard(results):
    out = np.empty((B, S, H), dtype=np.float32)
    for c in range(NCORES):
        b, g = c // 2, c % 2
        r = np.asarray(results[c]["out"], dtype=np.float32)  # [8, 65, 2048]
        ctxs = r[:, 0:HD, :] / r[:, HD:HD + 1, :]
        blk = ctxs.transpose(2, 0, 1).reshape(S, HG)
        out[b, :, g * HG:(g + 1) * HG] = blk
    return out


def kernel(hidden_states, Wq, bq, Wk, bk, Wv, bv):
    in_maps = make_in_maps(hidden_states, Wq, bq, Wk, bk, Wv, bv)
    res = run(in_maps, trace=False)
    return unshard(res.results)
